# revision 54
# baseline (speedup 1.0000x reference)
"""Trainium2 Bass kernel for nn_DeepAttention (pairwise-MLP attention).

Per-batch computation (B=8, S=512, T=64):
    hq = q @ W1[:T]; hk = k @ W1[T:]            # [S, 80]
    h  = relu(hq[i] + hk[j] + b1)               # per (i, j) pair, [80]
    h2 = relu(h @ W2 + b2)                      # [60]
    logits[i, j] = h2 @ W3 (+ b3; cancels in softmax)
    attn = softmax(where(mask, logits / 8, -1e16), axis=j)
    out  = attn @ v

Sharding: data-parallel over batch, one batch element per NeuronCore (8 cores),
weights replicated, no cross-core communication.

Device strategy per core:
  * hqT [80, S], hkT+b1 [80, S] via PE transposes + W1 matmuls (fp32),
    pipelined in 128-column chunks so the main loop starts early.
  * Queries processed in 16 "quarters" of 32 rows; causality limits the
    j-extent of quarter qq to Wq = 32*(qq+1) columns.
  * Per query pair (i0, i1): build h columns with one fused add+relu
    tensor_scalar per query (bf16 out) — even queries on DVE, odd queries
    on the otherwise-idle GPSIMD engine — written as slices of a
    group-wide tile. One W2 matmul per partition band (bf16, K=80, M=64,
    bands 0:64 / 64:128 run concurrently via PE column tiling) covers all
    pairs packed into the PSUM bank.
  * relu+b2 (PSUM -> bf16 SBUF) on ScalarE, every 4th group on DVE.
  * W3 stage: per pair one "selector" matmul lhsT [128, 32] holding W3 in
    rows 0:60 -> column 2r and rows 64:124 -> column 2r+1; 16 pairs
    accumulate into one PSUM tile [32, Wq] = the quarter's logits.
  * Masked softmax: memset -1e16 (GPSIMD), copy_predicated from PSUM where
    mask!=0, then uncentered exp with fused 1/TEMP scale + running row sum
    (ScalarE; logits/TEMP is O(1) so no max-subtraction is needed and
    masked lanes underflow to exactly 0), reciprocal + scale -> attn bf16.
  * out: PE-transpose attn 128-chunks, matmul against v (bf16) with PSUM
    accumulation over chunks.
"""

import os
import sys
import numpy as np
import ml_dtypes

for _p in ("/opt/trn_rl_repo", "/root/.axon_site/_ro/trn_rl_repo"):
    if _p not in sys.path and os.path.isdir(_p):
        sys.path.insert(0, _p)

from contextlib import ExitStack

import concourse.bass as bass
import concourse.tile as tile
from concourse import bacc, mybir
from concourse.bass_utils import run_bass_kernel_spmd

B, S, T = 8, 512, 64
F1, F2 = 80, 60
NCORES = 8
TEMP = 8.0
NEG = -1e16
P = 128

f32 = mybir.dt.float32
bf16 = mybir.dt.bfloat16
u8 = mybir.dt.uint8

_NC_CACHE = {}


def _build_module():
    nc = bacc.Bacc(None, target_bir_lowering=False)

    io = {}
    io["q"] = nc.declare_dram_parameter("q", [S, T], f32, isOutput=False)
    io["k"] = nc.declare_dram_parameter("k", [S, T], f32, isOutput=False)
    io["v"] = nc.declare_dram_parameter("v", [S, T], f32, isOutput=False)
    io["mask"] = nc.declare_dram_parameter("mask", [S, S], u8, isOutput=False)
    io["w1a"] = nc.declare_dram_parameter("w1a", [T, F1], f32, isOutput=False)
    io["w1b"] = nc.declare_dram_parameter("w1b", [T, F1], f32, isOutput=False)
    io["b1"] = nc.declare_dram_parameter("b1", [F1, 1], f32, isOutput=False)
    io["w2p"] = nc.declare_dram_parameter("w2p", [F1, 64], bf16, isOutput=False)
    io["b2s"] = nc.declare_dram_parameter("b2s", [P, 1], f32, isOutput=False)
    io["w3sel"] = nc.declare_dram_parameter("w3sel", [P, 16, 32], bf16, isOutput=False)
    io["id128"] = nc.declare_dram_parameter("id128", [P, P], f32, isOutput=False)
    io["id32"] = nc.declare_dram_parameter("id32", [32, 32], bf16, isOutput=False)
    io["out"] = nc.declare_dram_parameter("out", [S, T], f32, isOutput=True)
    io["attn"] = nc.declare_dram_parameter("attn", [S, S], bf16, isOutput=True)

    with ExitStack() as ctx:
        tc = ctx.enter_context(tile.TileContext(nc))
        _body(ctx, tc, io)
    return nc


def _body(ctx, tc, io):
    nc = tc.nc
    AF = mybir.ActivationFunctionType
    OP = mybir.AluOpType

    singles = ctx.enter_context(tc.tile_pool(name="singles", bufs=1))
    stage = ctx.enter_context(tc.tile_pool(name="stage", bufs=2))
    hp = ctx.enter_context(tc.tile_pool(name="hp", bufs=8))
    h2p = ctx.enter_context(tc.tile_pool(name="h2p", bufs=4))
    sup = ctx.enter_context(tc.tile_pool(name="sup", bufs=3))
    atp = ctx.enter_context(tc.tile_pool(name="atp", bufs=3))
    aTp = ctx.enter_context(tc.tile_pool(name="aTp", bufs=3))
    mkp = ctx.enter_context(tc.tile_pool(name="mkp", bufs=3))
    stp = ctx.enter_context(tc.tile_pool(name="stp", bufs=8))
    otp = ctx.enter_context(tc.tile_pool(name="otp", bufs=2))

    ps_h2 = ctx.enter_context(tc.tile_pool(name="ps_h2", bufs=5, space="PSUM"))
    ps_sc = ctx.enter_context(tc.tile_pool(name="ps_sc", bufs=1, space="PSUM"))
    ps_T = ctx.enter_context(tc.tile_pool(name="ps_T", bufs=1, space="PSUM"))
    ps_ov = ctx.enter_context(tc.tile_pool(name="ps_ov", bufs=1, space="PSUM"))

    # ---- constants / weights to SBUF.
    # Dispatched on ScalarE's DMA queue so the q/k chunk-0 loads (sync
    # queue) are not delayed behind eight weight-table transfers. ----
    id128_t = singles.tile([P, P], f32)
    nc.scalar.dma_start(out=id128_t, in_=io["id128"][:, :])
    w1a_t = singles.tile([T, F1], f32)
    nc.scalar.dma_start(out=w1a_t, in_=io["w1a"][:, :])
    w1b_t = singles.tile([T, F1], f32)
    nc.scalar.dma_start(out=w1b_t, in_=io["w1b"][:, :])
    b1_t = singles.tile([F1, 1], f32)
    nc.scalar.dma_start(out=b1_t, in_=io["b1"][:, :])
    w2p_t = singles.tile([F1, 64], bf16)
    nc.scalar.dma_start(out=w2p_t, in_=io["w2p"][:, :])
    b2s_t = singles.tile([P, 1], f32)
    nc.scalar.dma_start(out=b2s_t, in_=io["b2s"][:, :])
    w3s_t = singles.tile([P, 16, 32], bf16)
    nc.scalar.dma_start(out=w3s_t, in_=io["w3sel"][:, :, :])
    id32_t = singles.tile([32, 32], bf16)
    nc.scalar.dma_start(out=id32_t, in_=io["id32"][:, :])

    # ---- qT / kT via PE transposes; hqT, hkT(+b1) projections.
    # Chunked by 128 columns so quarter 0 only waits on chunk 0. ----
    qT = singles.tile([T, S], f32)
    kT = singles.tile([T, S], f32)
    hqT = singles.tile([F1, S], f32)
    hkT = singles.tile([F1, S], bf16)  # hk + b1, bf16 for fast DVE reads
    vst = singles.tile([P, S // P, T], f32)
    vloc = singles.tile([P, S // P, T], bf16)
    for c in range(S // P):
        sl = slice(c * P, (c + 1) * P)
        for src, dst, w_t, proj, projdst in (
            (io["q"], qT, w1a_t, "q", hqT),
            (io["k"], kT, w1b_t, "k", hkT),
        ):
            st = stage.tile([P, T], f32, tag="qk_stage", name="qk_stage")
            nc.sync.dma_start(out=st, in_=src[sl, :])
            pt = ps_h2.tile([T, P], f32, tag="ph2", name="qkT_ps")
            nc.tensor.transpose(pt, st, id128_t)
            nc.vector.tensor_copy(out=dst[:, sl], in_=pt)
            pp = ps_h2.tile([F1, P], f32, tag="ph2", name="proj_ps")
            nc.tensor.matmul(pp, w_t, dst[:, sl], start=True, stop=True)
            if proj == "q":
                nc.vector.tensor_copy(out=projdst[:, sl], in_=pp)
            else:
                nc.scalar.activation(
                    out=projdst[:, sl], in_=pp,
                    func=AF.Identity, bias=b1_t, scale=1.0,
                )
        nc.sync.dma_start(out=vst[:, c, :], in_=io["v"][sl, :])
        nc.scalar.copy(out=vloc[:, c, :], in_=vst[:, c, :])

    # ---- main loop over 16 query quarters ----
    for qq in range(16):
        Wq = 32 * (qq + 1)
        ibase = 32 * qq

        mask_t = mkp.tile([32, 512], u8, tag="mask", name="mask_t")
        nc.sync.dma_start(
            out=mask_t[:, :Wq], in_=io["mask"][ibase : ibase + 32, 0:Wq]
        )

        sc_ps = ps_sc.tile([32, 512], f32, tag="psc", name="sc_ps")

        G = max(1, 512 // Wq)  # pairs packed per h2 PSUM bank
        gidx = 0
        r = 0
        while r < 16:
            gpairs = list(range(r, min(r + G, 16)))
            width = len(gpairs) * Wq
            h2ps = ps_h2.tile([P, 512], f32, tag="ph2", name="h2ps")
            h2sb = h2p.tile([P, 512], bf16, tag="h2", name="h2sb")
            h0g = hp.tile([F1, 512], bf16, tag="h0", name="h0g")
            h1g = hp.tile([F1, 512], bf16, tag="h1", name="h1g")
            infos = []
            for gi, rr in enumerate(gpairs):
                i0 = ibase + 2 * rr
                i1 = i0 + 1
                off = gi * Wq
                nc.vector.tensor_scalar(
                    out=h0g[:, off : off + Wq],
                    in0=hkT[:, :Wq],
                    scalar1=hqT[:, i0 : i0 + 1],
                    scalar2=0.0,
                    op0=OP.add,
                    op1=OP.max,
                )
                nc.gpsimd.tensor_scalar(
                    out=h1g[:, off : off + Wq],
                    in0=hkT[:, :Wq],
                    scalar1=hqT[:, i1 : i1 + 1],
                    scalar2=0.0,
                    op0=OP.add,
                    op1=OP.max,
                )
                infos.append((rr, off))
                # W2 matmuls in sub-batches of <=4 pairs: PE starts
                # streaming while later h-builds of the group are still
                # running (one giant matmul would gate on the slowest
                # build of all 16 pairs in the early quarters)
                if gi % 4 == 3 or gi == len(gpairs) - 1:
                    sb_lo = (gi - gi % 4) * Wq
                    sb_hi = off + Wq
                    nc.tensor.matmul(
                        h2ps[0:64, sb_lo:sb_hi], w2p_t, h0g[:, sb_lo:sb_hi],
                        start=True, stop=True,
                    )
                    nc.tensor.matmul(
                        h2ps[64:128, sb_lo:sb_hi], w2p_t, h1g[:, sb_lo:sb_hi],
                        start=True, stop=True,
                    )
            if gidx % 4 == 3:
                # route every 4th relu group to DVE to unload ScalarE
                nc.vector.tensor_scalar(
                    out=h2sb[:, :width], in0=h2ps[:, :width],
                    scalar1=b2s_t, scalar2=0.0, op0=OP.add, op1=OP.max,
                )
            else:
                nc.scalar.activation(
                    out=h2sb[:, :width], in_=h2ps[:, :width],
                    func=AF.Relu, bias=b2s_t, scale=1.0,
                )
            gidx += 1
            for rr, off in infos:
                nc.tensor.matmul(
                    sc_ps[:, 0:Wq],
                    w3s_t[:, rr, :],
                    h2sb[:, off : off + Wq],
                    start=(rr == 0),
                    stop=(rr == 15),
                    skip_group_check=True,
                )
            r += G

        # ---- masked softmax over [32, Wq].
        # No max-subtraction: logits/TEMP is O(1) (bounded activations,
        # TEMP=8), masked lanes are -1e16 -> exp underflows to exactly 0,
        # so uncentered exp cannot overflow and softmax is shift-invariant.
        su = sup.tile([32, 512], f32, tag="su", name="su")
        nc.gpsimd.memset(su[:, :Wq], NEG)
        nc.vector.copy_predicated(
            out=su[:, :Wq], mask=mask_t[:, :Wq], data=sc_ps[:, :Wq]
        )
        at = atp.tile([32, 512], bf16, tag="attn", name="at")
        if Wq < 512:
            # tail-zero early: disjoint from exp's region, so the attn DMA
            # is not gated on a late memset
            nc.gpsimd.memset(at[:, Wq:], 0.0)
        sm = stp.tile([32, 1], f32, tag="sm", name="sm")
        nc.scalar.activation(
            out=at[:, :Wq], in_=su[:, :Wq],
            func=AF.Exp, bias=0.0, scale=1.0 / TEMP, accum_out=sm,
        )
        rv = stp.tile([32, 1], f32, tag="rv", name="rv")
        nc.vector.reciprocal(rv, sm)
        nc.vector.tensor_scalar_mul(at[:, :Wq], at[:, :Wq], rv)

        # ---- out rows: transpose attn chunks, matmul with v ----
        ov = ps_ov.tile([32, T], f32, tag="pov", name="ov")
        nch = (Wq + P - 1) // P
        # all chunk transposes of a quarter share one PSUM bank
        pT = ps_T.tile([P, 4, 32], bf16, tag="pT", name="pT")
        for c in range(nch):
            cs = min(P, Wq - P * c)
            nc.tensor.transpose(pT[0:cs, c, :], at[:, P * c : P * c + cs], id32_t)
            aT = aTp.tile([P, 32], bf16, tag="aT", name="aT")
            nc.vector.tensor_copy(out=aT[0:cs, :], in_=pT[0:cs, c, :])
            nc.tensor.matmul(
                ov,
                aT[0:cs, :],
                vloc[0:cs, c, :],
                start=(c == 0),
                stop=(c == nch - 1),
                skip_group_check=True,
            )
        nc.sync.dma_start(out=io["attn"][ibase : ibase + 32, :], in_=at)
        ot = otp.tile([32, T], f32, tag="ot", name="ot")
        nc.vector.tensor_copy(out=ot, in_=ov)
        nc.sync.dma_start(out=io["out"][ibase : ibase + 32, :], in_=ot)


def _host_inputs(q, k, v, mask, W1, b1, W2, b2, W3, b3):
    """Build the 8 per-core input maps (host-side weight packing only)."""
    bf = ml_dtypes.bfloat16
    w1a = np.ascontiguousarray(W1[:T]).astype(np.float32)
    w1b = np.ascontiguousarray(W1[T:]).astype(np.float32)
    b1c = np.ascontiguousarray(b1.reshape(F1, 1)).astype(np.float32)

    w2p = np.zeros((F1, 64), np.float32)
    w2p[:, :F2] = W2
    w2p = w2p.astype(bf)

    b2s = np.zeros((P, 1), np.float32)
    b2s[0:F2, 0] = b2
    b2s[64 : 64 + F2, 0] = b2

    w3sel = np.zeros((P, 16, 32), np.float32)
    for r in range(16):
        w3sel[0:F2, r, 2 * r] = W3[:, 0]
        w3sel[64 : 64 + F2, r, 2 * r + 1] = W3[:, 0]
    w3sel = w3sel.astype(bf)

    id128 = np.eye(P, dtype=np.float32)
    id32 = np.eye(32, dtype=np.float32).astype(bf)

    shared = {
        "w1a": w1a, "w1b": w1b, "b1": b1c, "w2p": w2p, "b2s": b2s,
        "w3sel": w3sel, "id128": id128, "id32": id32,
    }
    in_maps = []
    for c in range(NCORES):
        m = dict(shared)
        m["q"] = np.ascontiguousarray(q[c]).astype(np.float32)
        m["k"] = np.ascontiguousarray(k[c]).astype(np.float32)
        m["v"] = np.ascontiguousarray(v[c]).astype(np.float32)
        m["mask"] = np.ascontiguousarray(mask[c]).astype(np.uint8)
        in_maps.append(m)
    return in_maps


def get_module():
    if "nc" not in _NC_CACHE:
        nc = _build_module()
        if not nc.is_finalized():
            nc.finalize()
        _NC_CACHE["nc"] = nc
    return _NC_CACHE["nc"]


def kernel(q, k, v, mask, W1, b1, W2, b2, W3, b3):
    q = np.asarray(q); k = np.asarray(k); v = np.asarray(v)
    mask = np.asarray(mask)
    W1 = np.asarray(W1); b1 = np.asarray(b1)
    W2 = np.asarray(W2); b2 = np.asarray(b2)
    W3 = np.asarray(W3); b3 = np.asarray(b3)

    nc = get_module()
    in_maps = _host_inputs(q, k, v, mask, W1, b1, W2, b2, W3, b3)
    res = run_bass_kernel_spmd(nc, in_maps, list(range(NCORES)))
    outs = res.results
    out = np.stack([np.asarray(outs[c]["out"], dtype=np.float32) for c in range(NCORES)])
    attn = np.stack(
        [np.asarray(outs[c]["attn"]).astype(np.float32) for c in range(NCORES)]
    )
    return (out, attn)
